# revision 48
# baseline (speedup 1.0000x reference)
"""Multi-head attention block (QKV proj + softmax attention + out-proj +
residual + LayerNorm) on 8 TRN2 NeuronCores.

Sharding: core = (batch b, token-half g). Each core computes K/V for the
FULL 2048 tokens of its batch locally (all 8 heads) -- no collectives at
all -- and runs attention + out-proj + LayerNorm for its 1024 query
tokens. Host rotates tokens per core so the core's query tokens are
always columns 0..1023 (softmax over k is permutation-invariant).

Precision: weights and x are pre-quantized to fp8 (e4m3) on the host
(weights pre-scaled x32 for mantissa range); projection / PV / sum /
out-proj matmuls run in fp8 DoubleRow perf mode (256-deep contraction
per instruction = 2x bf16 rate). DoubleRow operand slices keep their two
k-planes >=16B apart (ISA rule 's3_lw_dual_fp8_restrictions'). Scores
(contraction 128) run plain fp8. PSUM accumulation is fp32; softmax
statistics and LayerNorm are fp32.

Schedule: attention runs q-slice (nq) outer, head inner, software-
pipelined (scores+exp issue 2 steps ahead of PV). The V projection is
fused with head-0's first pass -- PV for chunk pair cp only needs
V-chunks 2cp/2cp+1, so each pair of V-proj chunks is followed directly
by one attention step. Later heads' K/Q projections drip one unit per
step through the nq=0 pass; the out-projection + LN stats for q-chunks
0..3 drip through the nq=1 pass; LN Sqrts all run back-to-back in the
epilogue (one activation-table load).

Bias algebra (host-folded): bk drops out of softmax entirely (adds a
per-query constant to every logit); bv's contribution to ctx is exactly
bv (softmax weights sum to 1) so bv@Wo + bo folds into the residual
term; only bq survives in-kernel (added to Q). The residual is
pre-scaled x2048 to absorb all fp8 weight scales -- LayerNorm is
scale-invariant so only eps needs adjusting.
"""

import contextlib
import sys

if '/opt/trn_rl_repo' not in sys.path:
    sys.path.insert(0, '/opt/trn_rl_repo')

import ml_dtypes
import numpy as np

import concourse.bacc as bacc
import concourse.bass as bass
import concourse.bass_utils as bass_utils
import concourse.tile as tile
from concourse import mybir

B, T, D, H = 4, 2048, 1024, 8
DH = 128
TQ = T // 2
N_CORES = 8
NP = 4              # d-chunk pairs (contraction 1024 = 4 x 256)
CP = 8              # k-chunk pairs (2048 = 8 x 256)
NQ = 2              # 512-wide q slices per core
QC = TQ // 128
EPS = 1e-5
WS = 32.0           # fp8 weight pre-scale
XQS = 2048.0        # residual pre-scale (= ctx-scale 64 x Wo-scale 32)
SC_EXP = 1.0 / (WS * WS * float(np.sqrt(DH)))
EPS_S = XQS * XQS * EPS
F32 = mybir.dt.float32
BF16 = mybir.dt.bfloat16
FP8 = mybir.dt.float8e4
AF = mybir.ActivationFunctionType
ALU = mybir.AluOpType
DR = mybir.MatmulPerfMode.DoubleRow
BF = ml_dtypes.bfloat16
F8 = ml_dtypes.float8_e4m3


def _body(nc, tc, ap, es, apply_gb):
    xtm_d = ap['xtm']
    wq_d, wk_d, wv_d, wo_d = ap['wq'], ap['wk'], ap['wv'], ap['wo']
    bq_d, xq_d, gamma, beta, y = (ap['bq'], ap['xq'], ap['gamma'],
                                  ap['beta'], ap['y'])

    consts = es.enter_context(tc.tile_pool(name="consts", bufs=1))
    xt_pool = es.enter_context(tc.tile_pool(name="xt", bufs=1))
    w_pool = es.enter_context(tc.tile_pool(name="w", bufs=1))
    kts_pool = es.enter_context(tc.tile_pool(name="kts", bufs=1))
    qts_pool = es.enter_context(tc.tile_pool(name="qts", bufs=1))
    vp_pool = es.enter_context(tc.tile_pool(name="vp", bufs=1))
    ctx4_pool = es.enter_context(tc.tile_pool(name="ctx4", bufs=1))
    pt_pool = es.enter_context(tc.tile_pool(name="pt", bufs=12))
    sums_pool = es.enter_context(tc.tile_pool(name="sums", bufs=3))
    xr_pool = es.enter_context(tc.tile_pool(name="xr", bufs=4))
    y2_pool = es.enter_context(tc.tile_pool(name="y2", bufs=2))
    ln_pool = es.enter_context(tc.tile_pool(name="ln", bufs=8))
    y1_pool = es.enter_context(tc.tile_pool(name="y1p", bufs=8))
    ps512 = es.enter_context(tc.tile_pool(name="ps512", bufs=4,
                                          space="PSUM"))

    # ---- constants & weights (DMA order = need order) --------------------
    # dual-fp8 LDWEIGHTS needs the two k-planes >=16B apart; pad to 16
    ones2_t = consts.tile([128, 2, 16], FP8, tag="ones2")
    nc.vector.memset(ones2_t, 0.5)
    eps_t = consts.tile([128, 1], F32, tag="eps")
    nc.vector.memset(eps_t, EPS_S)
    bq_t = consts.tile([128, H], F32, tag="bq")
    nc.sync.dma_start(out=bq_t, in_=bq_d)
    ones2 = ones2_t[:, :, 0:1]

    def dma4(dst, src):
        nc.sync.dma_start(out=dst.rearrange("p a b c -> p (a b c)"),
                          in_=src.rearrange("p a b c -> p (a b c)"))

    def load_w(w_d, nm, shape):
        ts = []
        for p in range(NP):
            t = w_pool.tile(shape, FP8, tag=f"{nm}{p}", name=f"{nm}{p}")
            dma4(t, w_d[p])
            ts.append(t)
        return ts

    wk_t = load_w(wk_d, "wk", [128, H, 2, 128])
    xtm = []
    for p in range(NP):
        tm = xt_pool.tile([128, 4, 2, 512], FP8, tag=f"xtm{p}",
                          name=f"xtm{p}")
        dma4(tm, xtm_d[p])
        xtm.append(tm)
    wv_t = load_w(wv_d, "wv", [128, 2, 2, 512])
    wq_t = load_w(wq_d, "wq", [128, H, 2, 128])
    wo_t = load_w(wo_d, "wo", [128, 2, 2, 512])

    def xts(p, c):
        """V-proj stationary: [128, 2, 128] view of token chunk c."""
        return xtm[p][:, c // 4, :, (c % 4) * 128:(c % 4 + 1) * 128]

    gb = None
    if apply_gb:
        def bcast128(name, src):
            t = consts.tile([128, D], F32, tag=name, name=name)
            src_b = bass.AP(tensor=src.tensor, offset=src.offset,
                            ap=[[0, 128]] + src.ap)
            nc.sync.dma_start(out=t, in_=src_b)
            return t
        gb = [bcast128("gamma_b", gamma), bcast128("beta_b", beta)]

    kts = [kts_pool.tile([128, T], FP8, tag=f"kts{h}", name=f"kts{h}")
           for h in range(H)]
    qts = [qts_pool.tile([128, TQ], FP8, tag=f"qts{h}", name=f"qts{h}")
           for h in range(H)]
    vp = [vp_pool.tile([128, H, 2, 128], FP8, tag=f"vp{c}", name=f"vp{c}")
          for c in range(CP)]
    ctx4 = [ctx4_pool.tile([128, QC, 2, 128], FP8, tag=f"ctx{p}",
                           name=f"ctx{p}")
            for p in range(NP)]

    # ---- projection helpers ----------------------------------------------
    def proj_unit_K(h, nt):
        nsl = slice(nt * 512, (nt + 1) * 512)
        pp = ps512.tile([128, 512], F32, tag="ps", name="ppk")
        for p in range(NP):
            nc.tensor.matmul(pp, wk_t[p][:, h], xtm[p][:, nt],
                             start=(p == 0), stop=(p == NP - 1),
                             perf_mode=DR)
        nc.vector.tensor_copy(out=kts[h][:, nsl], in_=pp)

    def proj_unit_Q(h, nt):
        nsl = slice(nt * 512, (nt + 1) * 512)
        pp = ps512.tile([128, 512], F32, tag="ps", name="ppq")
        for p in range(NP):
            nc.tensor.matmul(pp, wq_t[p][:, h], xtm[p][:, nt],
                             start=(p == 0), stop=(p == NP - 1),
                             perf_mode=DR)
        nc.vector.tensor_scalar(out=qts[h][:, nsl], in0=pp,
                                scalar1=bq_t[:, h:h + 1], scalar2=None,
                                op0=ALU.add)

    def head_tasks(h):
        return ([lambda nt=nt: proj_unit_K(h, nt) for nt in range(4)]
                + [lambda nt=nt: proj_unit_Q(h, nt) for nt in range(2)])

    # ---- phase A: head-0 projections -------------------------------------
    for t in head_tasks(0):
        t()

    # score/exp issue machinery (runs ahead of the PV consumer)
    steps = [(nq, h, cp) for nq in range(NQ) for h in range(H)
             for cp in range(CP)]
    pt_q = {}
    cursor = [0]

    def issue_scores():
        i = cursor[0]
        nq, h, cp = steps[i]
        nsl = slice(nq * 512, (nq + 1) * 512)
        pt = pt_pool.tile([128, 2, 512], FP8, tag="pt", name="pt")
        sps = []
        for j in range(2):
            kc = cp * 2 + j
            s_ps = ps512.tile([128, 512], F32, tag="ps", name="s_ps")
            nc.tensor.matmul(s_ps, kts[h][:, kc * 128:(kc + 1) * 128],
                             qts[h][:, nsl], start=True, stop=True)
            sps.append(s_ps)
        for j in range(2):
            nc.scalar.activation(out=pt[:, j, :], in_=sps[j], func=AF.Exp,
                                 scale=SC_EXP)
        pt_q[i] = pt
        cursor[0] += 1

    # ---- phase C/D machinery ---------------------------------------------
    xr = {}

    def fetch_xr(qc):
        t = xr_pool.tile([128, D], F32, tag="xr", name="xr")
        nc.sync.dma_start(out=t, in_=xq_d[qc * 128:(qc + 1) * 128, :])
        xr[qc] = t

    def normalize(nq, h, ctx_ps, sum_ps):
        rsum = sums_pool.tile([1, 512], F32, tag="rsum", name="rsum")
        nc.vector.reciprocal_approx_fast(out=rsum, in_=sum_ps)
        rsum_b = sums_pool.tile([128, 512], F32, tag="rsum_b",
                                name="rsum_b")
        nc.gpsimd.partition_broadcast(rsum_b, rsum, channels=128)
        nc.vector.tensor_mul(
            out=ctx4[h // 2][:, 4 * nq:4 * nq + 4, h % 2, :],
            in0=ctx_ps.rearrange("p (a b) -> p a b", b=128),
            in1=rsum_b.rearrange("p (a b) -> p a b", b=128))

    y1s, mvs = {}, {}

    def outproj_stage1(qc):
        """Out-proj matmuls + residual + LN stats (no ScalarE table use
        so it can drip between EXPs without activation-table thrash)."""
        if qc + 4 < QC:
            fetch_xr(qc + 4)
        y1 = y1_pool.tile([128, D], F32, tag="y1", name="y1")
        for n2 in range(2):
            n2sl = slice(n2 * 512, (n2 + 1) * 512)
            pp = ps512.tile([128, 512], F32, tag="ps", name="ppo")
            for p in range(NP):
                nc.tensor.matmul(pp, ctx4[p][:, qc], wo_t[p][:, n2],
                                 start=(p == 0), stop=(p == NP - 1),
                                 perf_mode=DR)
            nc.vector.tensor_add(out=y1[:, n2sl], in0=pp,
                                 in1=xr[qc][:, n2sl])
        xr.pop(qc)
        stats = ln_pool.tile([128, 2, 6], F32, tag="stats", name="stats")
        y1g = y1.rearrange("p (n f) -> p n f", f=512)
        nc.vector.bn_stats(out=stats[:, 0, :], in_=y1g[:, 0, :])
        nc.vector.bn_stats(out=stats[:, 1, :], in_=y1g[:, 1, :])
        mv = ln_pool.tile([128, 2], F32, tag="mv", name="mv")
        nc.vector.bn_aggr(out=mv, in_=stats)
        y1s[qc], mvs[qc] = y1, mv

    def outproj_stage2(qc):
        """Sqrt + normalize + store. Phase D has no EXPs, so Sqrts keep
        the activation table cached; the apply alternates ScalarE/DVE."""
        qs = slice(qc * 128, (qc + 1) * 128)
        y1, mv = y1s.pop(qc), mvs.pop(qc)
        std = ln_pool.tile([128, 1], F32, tag="std", name="std")
        nc.scalar.activation(out=std, in_=mv[:, 1:2], func=AF.Sqrt,
                             bias=eps_t)
        rstd = ln_pool.tile([128, 1], F32, tag="rstd", name="rstd")
        nc.vector.reciprocal(out=rstd, in_=std)
        y2 = y2_pool.tile([128, D], F32, tag="y2", name="y2")
        if apply_gb:
            nc.vector.tensor_scalar(out=y2, in0=y1, scalar1=mv[:, 0:1],
                                    scalar2=rstd, op0=ALU.subtract,
                                    op1=ALU.mult)
            if apply_gb:
                nc.vector.tensor_mul(out=y2, in0=y2, in1=gb[0])
                nc.vector.tensor_add(out=y2, in0=y2, in1=gb[1])
        else:
            # y2 = Identity(y1 * rstd + (-mu * rstd)) on the idle ScalarE
            nmr = ln_pool.tile([128, 1], F32, tag="nmr", name="nmr")
            nc.vector.tensor_scalar(out=nmr, in0=mv[:, 0:1], scalar1=rstd,
                                    scalar2=-1.0, op0=ALU.mult,
                                    op1=ALU.mult)
            nc.scalar.activation(out=y2, in_=y1, func=AF.Identity,
                                 bias=nmr, scale=rstd)
        nc.sync.dma_start(out=y[qs, :], in_=y2)

    # ---- phase B: V-proj interleaved with head-0 scores ------------------
    with tc.tile_pool(name="psv", bufs=2, space="PSUM") as psV:
        for c in range(2 * CP):
            ppv = psV.tile([128, D], F32, tag="psv", name="ppv")
            for half in range(2):
                n2sl = slice(half * 512, (half + 1) * 512)
                for p in range(NP):
                    nc.tensor.matmul(ppv[:, n2sl], xts(p, c),
                                     wv_t[p][:, half],
                                     start=(p == 0),
                                     stop=(p == NP - 1), perf_mode=DR)
            nc.vector.tensor_copy(
                out=vp[c // 2][:, :, c % 2, :],
                in_=ppv.rearrange("p (h m) -> p h m", m=128))
            if cursor[0] < CP:
                issue_scores()

    for qc in range(4):
        fetch_xr(qc)

    # ---- phase C: attention (out-proj stage1 drips through pass 1) -------
    drip = []
    for h in range(1, H):
        drip.extend(head_tasks(h))
    post = [lambda qc=qc: outproj_stage1(qc) for qc in range(4)]

    with tc.tile_pool(name="ctxps", bufs=2, space="PSUM") as ctx_pool, \
         tc.tile_pool(name="sumps", bufs=2, space="PSUM") as sum_pool:
        state = {}

        def attention_step(i):
            nq, h, cp = steps[i]
            while cursor[0] <= min(i + 2, len(steps) - 1):
                issue_scores()
            if cp == 0:
                state['ctx'] = ctx_pool.tile([128, 512], F32, tag="ctx",
                                             name="ctx_ps")
                state['sum'] = sum_pool.tile([1, 512], F32, tag="sum",
                                             name="sum_ps")
            pt = pt_q.pop(i)
            nc.tensor.matmul(state['ctx'], vp[cp][:, h], pt,
                             start=(cp == 0), stop=(cp == CP - 1),
                             perf_mode=DR)
            nc.tensor.matmul(state['sum'], ones2, pt,
                             start=(cp == 0), stop=(cp == CP - 1),
                             perf_mode=DR)
            if nq == 0 and drip:
                drip.pop(0)()
            elif nq == 1 and post and (i - CP * H) % 12 == 6:
                post.pop(0)()
            if cp == CP - 1:
                normalize(nq, h, state['ctx'], state['sum'])

        for i in range(len(steps)):
            attention_step(i)

    # ---- phase D: remaining out-projection q-chunks + LN epilogue --------
    # interleave: qc 0..3 finished stage1 during the nq=1 pass, so their
    # epilogue starts immediately while qc 4..7's matmuls run
    while post:
        post.pop(0)()
    for k in range(4):
        outproj_stage1(4 + k)
        outproj_stage2(k)
    for qc in range(4, QC):
        outproj_stage2(qc)


def build(apply_gb=True):
    nc = bacc.Bacc("TRN2", target_bir_lowering=False, debug=False,
                   enable_asserts=False, num_devices=N_CORES)
    ap = {}
    ap['xtm'] = nc.dram_tensor("xtm", [NP, 128, 4, 2, 512], FP8,
                               kind="ExternalInput").ap()
    for nm in ('wq', 'wk'):
        ap[nm] = nc.dram_tensor(nm, [NP, 128, H, 2, 128], FP8,
                                kind="ExternalInput").ap()
    for nm in ('wv', 'wo'):
        ap[nm] = nc.dram_tensor(nm, [NP, 128, 2, 2, 512], FP8,
                                kind="ExternalInput").ap()
    ap['bq'] = nc.dram_tensor("bq", [128, H], F32, kind="ExternalInput").ap()
    ap['xq'] = nc.dram_tensor("xq", [TQ, D], F32, kind="ExternalInput").ap()
    ap['gamma'] = nc.dram_tensor("gamma", [D], F32,
                                 kind="ExternalInput").ap()
    ap['beta'] = nc.dram_tensor("beta", [D], F32, kind="ExternalInput").ap()
    ap['y'] = nc.dram_tensor("y", [TQ, D], F32, kind="ExternalOutput").ap()

    with tile.TileContext(nc) as tc, contextlib.ExitStack() as es:
        _body(nc, tc, ap, es, apply_gb)
    nc.compile()
    return nc


def _pack_pairs(w8, inner):
    """[D, N] fp8 -> [NP, 128, N//inner, 2, inner]: row (2p+j)*128+r,
    col (o*inner+m) lands at [p, r, o, j, m] (k-plane pairs contiguous)."""
    n = w8.shape[1]
    return np.ascontiguousarray(
        w8.reshape(NP, 2, 128, n // inner, inner).transpose(0, 2, 3, 1, 4))


def make_in_maps(inputs):
    """Per-core input maps; x token-rotated so q tokens come first."""
    f32 = {k: np.asarray(v, dtype=np.float32) for k, v in inputs.items()}

    def w8(nm):
        return (f32[nm] * WS).astype(F8)

    shared = {
        'wq': _pack_pairs(w8('Wq'), 128),
        'wk': _pack_pairs(w8('Wk'), 128),
        'wv': _pack_pairs(w8('Wv'), 512),
        'wo': _pack_pairs(w8('Wo'), 512),
        'bq': np.ascontiguousarray(
            (WS * f32['bq']).reshape(H, 128).T.astype(np.float32)),
        'gamma': f32['gamma'],
        'beta': f32['beta'],
    }
    resid_c = f32['bo'] + f32['bv'] @ f32['Wo']
    x = f32['x']
    in_maps = []
    for core in range(N_CORES):
        b, g = divmod(core, 2)
        xr = np.roll(x[b], -TQ * g, axis=0)
        xt8 = xr.T.astype(F8)  # [D, T]
        in_maps.append({
            'xtm': _pack_pairs(xt8, 512),
            'xq': np.ascontiguousarray(XQS * (xr[:TQ] + resid_c)),
            **shared})
    return in_maps


_NC = None
_NC_GB = None


def kernel(**inputs):
    global _NC, _NC_GB
    apply_gb = not (np.all(np.asarray(inputs['gamma']) == 1.0)
                    and np.all(np.asarray(inputs['beta']) == 0.0))
    if _NC is None or _NC_GB != apply_gb:
        _NC = build(apply_gb)
        _NC_GB = apply_gb
    in_maps = make_in_maps(inputs)
    res = bass_utils.run_bass_kernel_spmd(_NC, in_maps,
                                          core_ids=list(range(N_CORES)))
    out = np.empty((B, T, D), dtype=np.float32)
    for core in range(N_CORES):
        b, g = divmod(core, 2)
        out[b, TQ * g:TQ * (g + 1)] = res.results[core]['y']
    return out


# revision 50
# speedup vs baseline: 1.0354x; 1.0354x over previous
"""Multi-head attention block (QKV proj + softmax attention + out-proj +
residual + LayerNorm) on 8 TRN2 NeuronCores.

Sharding: core = (batch b, token-half g). Each core computes K/V for the
FULL 2048 tokens of its batch locally (all 8 heads) -- no collectives at
all -- and runs attention + out-proj + LayerNorm for its 1024 query
tokens. Host rotates tokens per core so the core's query tokens are
always columns 0..1023 (softmax over k is permutation-invariant).

Precision: weights and x are pre-quantized to fp8 (e4m3) on the host
(weights pre-scaled x32 for mantissa range); projection / PV / sum /
out-proj matmuls run in fp8 DoubleRow perf mode (256-deep contraction
per instruction = 2x bf16 rate). DoubleRow operand slices keep their two
k-planes >=16B apart (ISA rule 's3_lw_dual_fp8_restrictions'). Scores
(contraction 128) run plain fp8. PSUM accumulation is fp32; softmax
statistics and LayerNorm are fp32.

Schedule: attention runs q-slice (nq) outer, head inner, software-
pipelined (scores+exp issue 2 steps ahead of PV). The V projection is
fused with head-0's first pass -- PV for chunk pair cp only needs
V-chunks 2cp/2cp+1, so each pair of V-proj chunks is followed directly
by one attention step. Later heads' K/Q projections drip one unit per
step through the nq=0 pass; the out-projection + LN stats for q-chunks
0..3 drip through the nq=1 pass; LN Sqrts all run back-to-back in the
epilogue (one activation-table load).

Bias algebra (host-folded): bk drops out of softmax entirely (adds a
per-query constant to every logit); bv's contribution to ctx is exactly
bv (softmax weights sum to 1) so bv@Wo + bo folds into the residual
term; only bq survives in-kernel (added to Q). The residual is
pre-scaled x2048 to absorb all fp8 weight scales -- LayerNorm is
scale-invariant so only eps needs adjusting.
"""

import contextlib
import sys

if '/opt/trn_rl_repo' not in sys.path:
    sys.path.insert(0, '/opt/trn_rl_repo')

import ml_dtypes
import numpy as np

import concourse.bacc as bacc
import concourse.bass as bass
import concourse.bass_utils as bass_utils
import concourse.tile as tile
from concourse import mybir

B, T, D, H = 4, 2048, 1024, 8
DH = 128
TQ = T // 2
N_CORES = 8
NP = 4              # d-chunk pairs (contraction 1024 = 4 x 256)
CP = 8              # k-chunk pairs (2048 = 8 x 256)
NQ = 2              # 512-wide q slices per core
QC = TQ // 128
EPS = 1e-5
WS = 32.0           # fp8 weight pre-scale
XQS = 2048.0        # residual pre-scale (= ctx-scale 64 x Wo-scale 32)
SC_EXP = 1.0 / (WS * WS * float(np.sqrt(DH)))
EPS_S = XQS * XQS * EPS
F32 = mybir.dt.float32
BF16 = mybir.dt.bfloat16
FP8 = mybir.dt.float8e4
AF = mybir.ActivationFunctionType
ALU = mybir.AluOpType
DR = mybir.MatmulPerfMode.DoubleRow
BF = ml_dtypes.bfloat16
F8 = ml_dtypes.float8_e4m3


def _body(nc, tc, ap, es, apply_gb):
    xtm_d = ap['xtm']
    wq_d, wk_d, wv_d, wo_d = ap['wq'], ap['wk'], ap['wv'], ap['wo']
    bq_d, xq_d, gamma, beta, y = (ap['bq'], ap['xq'], ap['gamma'],
                                  ap['beta'], ap['y'])

    consts = es.enter_context(tc.tile_pool(name="consts", bufs=1))
    xt_pool = es.enter_context(tc.tile_pool(name="xt", bufs=1))
    w_pool = es.enter_context(tc.tile_pool(name="w", bufs=1))
    kts_pool = es.enter_context(tc.tile_pool(name="kts", bufs=1))
    qts_pool = es.enter_context(tc.tile_pool(name="qts", bufs=1))
    vp_pool = es.enter_context(tc.tile_pool(name="vp", bufs=1))
    ctx4_pool = es.enter_context(tc.tile_pool(name="ctx4", bufs=1))
    pt_pool = es.enter_context(tc.tile_pool(name="pt", bufs=12))
    sums_pool = es.enter_context(tc.tile_pool(name="sums", bufs=3))
    xr_pool = es.enter_context(tc.tile_pool(name="xr", bufs=4))
    y2_pool = es.enter_context(tc.tile_pool(name="y2", bufs=2))
    ln_pool = es.enter_context(tc.tile_pool(name="ln", bufs=8))
    y1_pool = es.enter_context(tc.tile_pool(name="y1p", bufs=8))
    ps512 = es.enter_context(tc.tile_pool(name="ps512", bufs=4,
                                          space="PSUM"))

    # ---- constants & weights (DMA order = need order) --------------------
    # dual-fp8 LDWEIGHTS needs the two k-planes >=16B apart; pad to 16
    ones2_t = consts.tile([128, 2, 16], FP8, tag="ones2")
    nc.vector.memset(ones2_t, 0.5)
    eps_t = consts.tile([128, 1], F32, tag="eps")
    nc.vector.memset(eps_t, EPS_S)
    bq_t = consts.tile([128, H], F32, tag="bq")
    nc.sync.dma_start(out=bq_t, in_=bq_d)
    ones2 = ones2_t[:, :, 0:1]

    def dma4(dst, src):
        nc.sync.dma_start(out=dst.rearrange("p a b c -> p (a b c)"),
                          in_=src.rearrange("p a b c -> p (a b c)"))

    def load_w(w_d, nm, shape):
        ts = []
        for p in range(NP):
            t = w_pool.tile(shape, FP8, tag=f"{nm}{p}", name=f"{nm}{p}")
            dma4(t, w_d[p])
            ts.append(t)
        return ts

    wk_t = load_w(wk_d, "wk", [128, H, 2, 128])
    xtm = []
    for p in range(NP):
        tm = xt_pool.tile([128, 4, 2, 512], FP8, tag=f"xtm{p}",
                          name=f"xtm{p}")
        dma4(tm, xtm_d[p])
        xtm.append(tm)
    wv_t = load_w(wv_d, "wv", [128, 2, 2, 512])
    wq_t = load_w(wq_d, "wq", [128, H, 2, 128])
    wo_t = load_w(wo_d, "wo", [128, 2, 2, 512])

    def xts(p, c):
        """V-proj stationary: [128, 2, 128] view of token chunk c."""
        return xtm[p][:, c // 4, :, (c % 4) * 128:(c % 4 + 1) * 128]

    gb = None
    if apply_gb:
        def bcast128(name, src):
            t = consts.tile([128, D], F32, tag=name, name=name)
            src_b = bass.AP(tensor=src.tensor, offset=src.offset,
                            ap=[[0, 128]] + src.ap)
            nc.sync.dma_start(out=t, in_=src_b)
            return t
        gb = [bcast128("gamma_b", gamma), bcast128("beta_b", beta)]

    kts = [kts_pool.tile([128, T], FP8, tag=f"kts{h}", name=f"kts{h}")
           for h in range(H)]
    qts = [qts_pool.tile([128, TQ], FP8, tag=f"qts{h}", name=f"qts{h}")
           for h in range(H)]
    vp = [vp_pool.tile([128, H, 2, 128], FP8, tag=f"vp{c}", name=f"vp{c}")
          for c in range(CP)]
    ctx4 = [ctx4_pool.tile([128, QC, 2, 128], FP8, tag=f"ctx{p}",
                           name=f"ctx{p}")
            for p in range(NP)]

    # ---- projection helpers ----------------------------------------------
    def proj_unit_K(h, nt):
        nsl = slice(nt * 512, (nt + 1) * 512)
        pp = ps512.tile([128, 512], F32, tag="ps", name="ppk")
        for p in range(NP):
            nc.tensor.matmul(pp, wk_t[p][:, h], xtm[p][:, nt],
                             start=(p == 0), stop=(p == NP - 1),
                             perf_mode=DR)
        nc.vector.tensor_copy(out=kts[h][:, nsl], in_=pp)

    def proj_unit_Q(h, nt):
        nsl = slice(nt * 512, (nt + 1) * 512)
        pp = ps512.tile([128, 512], F32, tag="ps", name="ppq")
        for p in range(NP):
            nc.tensor.matmul(pp, wq_t[p][:, h], xtm[p][:, nt],
                             start=(p == 0), stop=(p == NP - 1),
                             perf_mode=DR)
        nc.vector.tensor_scalar(out=qts[h][:, nsl], in0=pp,
                                scalar1=bq_t[:, h:h + 1], scalar2=None,
                                op0=ALU.add)

    def head_tasks(h):
        return ([lambda nt=nt: proj_unit_K(h, nt) for nt in range(4)]
                + [lambda nt=nt: proj_unit_Q(h, nt) for nt in range(2)])

    # ---- phase A: head-0 projections -------------------------------------
    for t in head_tasks(0):
        t()

    # score/exp issue machinery (runs ahead of the PV consumer)
    steps = [(nq, h, cp) for nq in range(NQ) for h in range(H)
             for cp in range(CP)]
    pt_q = {}
    cursor = [0]

    # In the nq=1 pass ScalarE paces the attention (2 EXPs/step > PE
    # work) while the DVE has slack, so alternating steps offload their
    # second exp chunk to the DVE as a Schraudolph fp8 exp:
    # bits8(e4m3) = round(logit * 8*log2e + 55.63)
    SCH_A = 8.0 * 1.4426950408889634 * SC_EXP
    SCH_B = 55.63
    I8 = mybir.dt.int8

    def issue_scores():
        i = cursor[0]
        nq, h, cp = steps[i]
        nsl = slice(nq * 512, (nq + 1) * 512)
        pt = pt_pool.tile([128, 2, 512], FP8, tag="pt", name="pt")
        sps = []
        for j in range(2):
            kc = cp * 2 + j
            s_ps = ps512.tile([128, 512], F32, tag="ps", name="s_ps")
            nc.tensor.matmul(s_ps, kts[h][:, kc * 128:(kc + 1) * 128],
                             qts[h][:, nsl], start=True, stop=True)
            sps.append(s_ps)
        nc.scalar.activation(out=pt[:, 0, :], in_=sps[0], func=AF.Exp,
                             scale=SC_EXP)
        if nq == 1 and i % 2 == 1:
            nc.vector.tensor_scalar(out=pt[:, 1, :].bitcast(I8),
                                    in0=sps[1], scalar1=SCH_A,
                                    scalar2=SCH_B, op0=ALU.mult,
                                    op1=ALU.add)
        else:
            nc.scalar.activation(out=pt[:, 1, :], in_=sps[1], func=AF.Exp,
                                 scale=SC_EXP)
        pt_q[i] = pt
        cursor[0] += 1

    # ---- phase C/D machinery ---------------------------------------------
    xr = {}

    def fetch_xr(qc):
        t = xr_pool.tile([128, D], F32, tag="xr", name="xr")
        nc.sync.dma_start(out=t, in_=xq_d[qc * 128:(qc + 1) * 128, :])
        xr[qc] = t

    def normalize(nq, h, ctx_ps, sum_ps):
        rsum = sums_pool.tile([1, 512], F32, tag="rsum", name="rsum")
        nc.vector.reciprocal_approx_fast(out=rsum, in_=sum_ps)
        rsum_b = sums_pool.tile([128, 512], F32, tag="rsum_b",
                                name="rsum_b")
        nc.gpsimd.partition_broadcast(rsum_b, rsum, channels=128)
        nc.vector.tensor_mul(
            out=ctx4[h // 2][:, 4 * nq:4 * nq + 4, h % 2, :],
            in0=ctx_ps.rearrange("p (a b) -> p a b", b=128),
            in1=rsum_b.rearrange("p (a b) -> p a b", b=128))

    y1s, mvs = {}, {}

    def outproj_stage1(qc):
        """Out-proj matmuls + residual + LN stats (no ScalarE table use
        so it can drip between EXPs without activation-table thrash)."""
        if qc + 4 < QC:
            fetch_xr(qc + 4)
        y1 = y1_pool.tile([128, D], F32, tag="y1", name="y1")
        for n2 in range(2):
            n2sl = slice(n2 * 512, (n2 + 1) * 512)
            pp = ps512.tile([128, 512], F32, tag="ps", name="ppo")
            for p in range(NP):
                nc.tensor.matmul(pp, ctx4[p][:, qc], wo_t[p][:, n2],
                                 start=(p == 0), stop=(p == NP - 1),
                                 perf_mode=DR)
            nc.vector.tensor_add(out=y1[:, n2sl], in0=pp,
                                 in1=xr[qc][:, n2sl])
        xr.pop(qc)
        stats = ln_pool.tile([128, 2, 6], F32, tag="stats", name="stats")
        y1g = y1.rearrange("p (n f) -> p n f", f=512)
        nc.vector.bn_stats(out=stats[:, 0, :], in_=y1g[:, 0, :])
        nc.vector.bn_stats(out=stats[:, 1, :], in_=y1g[:, 1, :])
        mv = ln_pool.tile([128, 2], F32, tag="mv", name="mv")
        nc.vector.bn_aggr(out=mv, in_=stats)
        y1s[qc], mvs[qc] = y1, mv

    def outproj_stage2(qc):
        """Sqrt + normalize + store. Phase D has no EXPs, so Sqrts keep
        the activation table cached; the apply alternates ScalarE/DVE."""
        qs = slice(qc * 128, (qc + 1) * 128)
        y1, mv = y1s.pop(qc), mvs.pop(qc)
        std = ln_pool.tile([128, 1], F32, tag="std", name="std")
        nc.scalar.activation(out=std, in_=mv[:, 1:2], func=AF.Sqrt,
                             bias=eps_t)
        rstd = ln_pool.tile([128, 1], F32, tag="rstd", name="rstd")
        nc.vector.reciprocal(out=rstd, in_=std)
        y2 = y2_pool.tile([128, D], F32, tag="y2", name="y2")
        if apply_gb or qc % 2 == 1:
            nc.vector.tensor_scalar(out=y2, in0=y1, scalar1=mv[:, 0:1],
                                    scalar2=rstd, op0=ALU.subtract,
                                    op1=ALU.mult)
            if apply_gb:
                nc.vector.tensor_mul(out=y2, in0=y2, in1=gb[0])
                nc.vector.tensor_add(out=y2, in0=y2, in1=gb[1])
        else:
            # y2 = Identity(y1 * rstd + (-mu * rstd)) on the idle ScalarE
            nmr = ln_pool.tile([128, 1], F32, tag="nmr", name="nmr")
            nc.vector.tensor_scalar(out=nmr, in0=mv[:, 0:1], scalar1=rstd,
                                    scalar2=-1.0, op0=ALU.mult,
                                    op1=ALU.mult)
            nc.scalar.activation(out=y2, in_=y1, func=AF.Identity,
                                 bias=nmr, scale=rstd)
        nc.sync.dma_start(out=y[qs, :], in_=y2)

    # ---- phase B: V-proj interleaved with head-0 scores ------------------
    with tc.tile_pool(name="psv", bufs=2, space="PSUM") as psV:
        for c in range(2 * CP):
            ppv = psV.tile([128, D], F32, tag="psv", name="ppv")
            for half in range(2):
                n2sl = slice(half * 512, (half + 1) * 512)
                for p in range(NP):
                    nc.tensor.matmul(ppv[:, n2sl], xts(p, c),
                                     wv_t[p][:, half],
                                     start=(p == 0),
                                     stop=(p == NP - 1), perf_mode=DR)
            nc.vector.tensor_copy(
                out=vp[c // 2][:, :, c % 2, :],
                in_=ppv.rearrange("p (h m) -> p h m", m=128))
            if cursor[0] < CP:
                issue_scores()

    for qc in range(4):
        fetch_xr(qc)

    # ---- phase C: attention (out-proj stage1 drips through pass 1) -------
    drip = []
    for h in range(1, H):
        drip.extend(head_tasks(h))
    post = [lambda qc=qc: outproj_stage1(qc) for qc in range(4)]

    with tc.tile_pool(name="ctxps", bufs=2, space="PSUM") as ctx_pool, \
         tc.tile_pool(name="sumps", bufs=2, space="PSUM") as sum_pool:
        state = {}

        def attention_step(i):
            nq, h, cp = steps[i]
            while cursor[0] <= min(i + 2, len(steps) - 1):
                issue_scores()
            if cp == 0:
                state['ctx'] = ctx_pool.tile([128, 512], F32, tag="ctx",
                                             name="ctx_ps")
                state['sum'] = sum_pool.tile([1, 512], F32, tag="sum",
                                             name="sum_ps")
            pt = pt_q.pop(i)
            nc.tensor.matmul(state['ctx'], vp[cp][:, h], pt,
                             start=(cp == 0), stop=(cp == CP - 1),
                             perf_mode=DR)
            nc.tensor.matmul(state['sum'], ones2, pt,
                             start=(cp == 0), stop=(cp == CP - 1),
                             perf_mode=DR)
            if nq == 0 and drip:
                drip.pop(0)()
            elif nq == 1 and post and (i - CP * H) % 12 == 6:
                post.pop(0)()
            if cp == CP - 1:
                normalize(nq, h, state['ctx'], state['sum'])

        for i in range(len(steps)):
            attention_step(i)

    # ---- phase D: remaining out-projection q-chunks + LN epilogue --------
    # interleave: qc 0..3 finished stage1 during the nq=1 pass, so their
    # epilogue starts immediately while qc 4..7's matmuls run
    while post:
        post.pop(0)()
    for k in range(4):
        outproj_stage1(4 + k)
        outproj_stage2(k)
    for qc in range(4, QC):
        outproj_stage2(qc)


def build(apply_gb=True):
    nc = bacc.Bacc("TRN2", target_bir_lowering=False, debug=False,
                   enable_asserts=False, num_devices=N_CORES)
    ap = {}
    ap['xtm'] = nc.dram_tensor("xtm", [NP, 128, 4, 2, 512], FP8,
                               kind="ExternalInput").ap()
    for nm in ('wq', 'wk'):
        ap[nm] = nc.dram_tensor(nm, [NP, 128, H, 2, 128], FP8,
                                kind="ExternalInput").ap()
    for nm in ('wv', 'wo'):
        ap[nm] = nc.dram_tensor(nm, [NP, 128, 2, 2, 512], FP8,
                                kind="ExternalInput").ap()
    ap['bq'] = nc.dram_tensor("bq", [128, H], F32, kind="ExternalInput").ap()
    ap['xq'] = nc.dram_tensor("xq", [TQ, D], F32, kind="ExternalInput").ap()
    ap['gamma'] = nc.dram_tensor("gamma", [D], F32,
                                 kind="ExternalInput").ap()
    ap['beta'] = nc.dram_tensor("beta", [D], F32, kind="ExternalInput").ap()
    ap['y'] = nc.dram_tensor("y", [TQ, D], F32, kind="ExternalOutput").ap()

    with tile.TileContext(nc) as tc, contextlib.ExitStack() as es:
        _body(nc, tc, ap, es, apply_gb)
    nc.compile()
    return nc


def _pack_pairs(w8, inner):
    """[D, N] fp8 -> [NP, 128, N//inner, 2, inner]: row (2p+j)*128+r,
    col (o*inner+m) lands at [p, r, o, j, m] (k-plane pairs contiguous)."""
    n = w8.shape[1]
    return np.ascontiguousarray(
        w8.reshape(NP, 2, 128, n // inner, inner).transpose(0, 2, 3, 1, 4))


def make_in_maps(inputs):
    """Per-core input maps; x token-rotated so q tokens come first."""
    f32 = {k: np.asarray(v, dtype=np.float32) for k, v in inputs.items()}

    def w8(nm):
        return (f32[nm] * WS).astype(F8)

    shared = {
        'wq': _pack_pairs(w8('Wq'), 128),
        'wk': _pack_pairs(w8('Wk'), 128),
        'wv': _pack_pairs(w8('Wv'), 512),
        'wo': _pack_pairs(w8('Wo'), 512),
        'bq': np.ascontiguousarray(
            (WS * f32['bq']).reshape(H, 128).T.astype(np.float32)),
        'gamma': f32['gamma'],
        'beta': f32['beta'],
    }
    resid_c = f32['bo'] + f32['bv'] @ f32['Wo']
    x = f32['x']
    in_maps = []
    for core in range(N_CORES):
        b, g = divmod(core, 2)
        xr = np.roll(x[b], -TQ * g, axis=0)
        xt8 = xr.T.astype(F8)  # [D, T]
        in_maps.append({
            'xtm': _pack_pairs(xt8, 512),
            'xq': np.ascontiguousarray(XQS * (xr[:TQ] + resid_c)),
            **shared})
    return in_maps


_NC = None
_NC_GB = None


def kernel(**inputs):
    global _NC, _NC_GB
    apply_gb = not (np.all(np.asarray(inputs['gamma']) == 1.0)
                    and np.all(np.asarray(inputs['beta']) == 0.0))
    if _NC is None or _NC_GB != apply_gb:
        _NC = build(apply_gb)
        _NC_GB = apply_gb
    in_maps = make_in_maps(inputs)
    res = bass_utils.run_bass_kernel_spmd(_NC, in_maps,
                                          core_ids=list(range(N_CORES)))
    out = np.empty((B, T, D), dtype=np.float32)
    for core in range(N_CORES):
        b, g = divmod(core, 2)
        out[b, TQ * g:TQ * (g + 1)] = res.results[core]['y']
    return out


# revision 51
# speedup vs baseline: 1.0374x; 1.0019x over previous
"""Multi-head attention block (QKV proj + softmax attention + out-proj +
residual + LayerNorm) on 8 TRN2 NeuronCores.

Sharding: core = (batch b, token-half g). Each core computes K/V for the
FULL 2048 tokens of its batch locally (all 8 heads) -- no collectives at
all -- and runs attention + out-proj + LayerNorm for its 1024 query
tokens. Host rotates tokens per core so the core's query tokens are
always columns 0..1023 (softmax over k is permutation-invariant).

Precision: weights and x are pre-quantized to fp8 (e4m3) on the host
(weights pre-scaled x32 for mantissa range); projection / PV / sum /
out-proj matmuls run in fp8 DoubleRow perf mode (256-deep contraction
per instruction = 2x bf16 rate). DoubleRow operand slices keep their two
k-planes >=16B apart (ISA rule 's3_lw_dual_fp8_restrictions'). Scores
(contraction 128) run plain fp8. PSUM accumulation is fp32; softmax
statistics and LayerNorm are fp32.

Schedule: attention runs q-slice (nq) outer, head inner, software-
pipelined (scores+exp issue 2 steps ahead of PV). The V projection is
fused with head-0's first pass -- PV for chunk pair cp only needs
V-chunks 2cp/2cp+1, so each pair of V-proj chunks is followed directly
by one attention step. Later heads' K/Q projections drip one unit per
step through the nq=0 pass; the out-projection + LN stats for q-chunks
0..3 drip through the nq=1 pass; LN Sqrts all run back-to-back in the
epilogue (one activation-table load).

Bias algebra (host-folded): bk drops out of softmax entirely (adds a
per-query constant to every logit); bv's contribution to ctx is exactly
bv (softmax weights sum to 1) so bv@Wo + bo folds into the residual
term; only bq survives in-kernel (added to Q). The residual is
pre-scaled x2048 to absorb all fp8 weight scales -- LayerNorm is
scale-invariant so only eps needs adjusting.
"""

import contextlib
import sys

if '/opt/trn_rl_repo' not in sys.path:
    sys.path.insert(0, '/opt/trn_rl_repo')

import ml_dtypes
import numpy as np

import concourse.bacc as bacc
import concourse.bass as bass
import concourse.bass_utils as bass_utils
import concourse.tile as tile
from concourse import mybir

B, T, D, H = 4, 2048, 1024, 8
DH = 128
TQ = T // 2
N_CORES = 8
NP = 4              # d-chunk pairs (contraction 1024 = 4 x 256)
CP = 8              # k-chunk pairs (2048 = 8 x 256)
NQ = 2              # 512-wide q slices per core
QC = TQ // 128
EPS = 1e-5
WS = 32.0           # fp8 weight pre-scale
XQS = 2048.0        # residual pre-scale (= ctx-scale 64 x Wo-scale 32)
SC_EXP = 1.0 / (WS * WS * float(np.sqrt(DH)))
EPS_S = XQS * XQS * EPS
F32 = mybir.dt.float32
BF16 = mybir.dt.bfloat16
FP8 = mybir.dt.float8e4
AF = mybir.ActivationFunctionType
ALU = mybir.AluOpType
DR = mybir.MatmulPerfMode.DoubleRow
BF = ml_dtypes.bfloat16
F8 = ml_dtypes.float8_e4m3


def _body(nc, tc, ap, es, apply_gb):
    xtm_d = ap['xtm']
    wq_d, wk_d, wv_d, wo_d = ap['wq'], ap['wk'], ap['wv'], ap['wo']
    bq_d, xq_d, gamma, beta, y = (ap['bq'], ap['xq'], ap['gamma'],
                                  ap['beta'], ap['y'])

    consts = es.enter_context(tc.tile_pool(name="consts", bufs=1))
    xt_pool = es.enter_context(tc.tile_pool(name="xt", bufs=1))
    w_pool = es.enter_context(tc.tile_pool(name="w", bufs=1))
    kts_pool = es.enter_context(tc.tile_pool(name="kts", bufs=1))
    qts_pool = es.enter_context(tc.tile_pool(name="qts", bufs=1))
    vp_pool = es.enter_context(tc.tile_pool(name="vp", bufs=1))
    ctx4_pool = es.enter_context(tc.tile_pool(name="ctx4", bufs=1))
    pt_pool = es.enter_context(tc.tile_pool(name="pt", bufs=12))
    sums_pool = es.enter_context(tc.tile_pool(name="sums", bufs=3))
    xr_pool = es.enter_context(tc.tile_pool(name="xr", bufs=4))
    y2_pool = es.enter_context(tc.tile_pool(name="y2", bufs=2))
    ln_pool = es.enter_context(tc.tile_pool(name="ln", bufs=8))
    y1_pool = es.enter_context(tc.tile_pool(name="y1p", bufs=8))
    ps512 = es.enter_context(tc.tile_pool(name="ps512", bufs=4,
                                          space="PSUM"))

    # ---- constants & weights (DMA order = need order) --------------------
    # dual-fp8 LDWEIGHTS needs the two k-planes >=16B apart; pad to 16
    ones2_t = consts.tile([128, 2, 16], FP8, tag="ones2")
    nc.vector.memset(ones2_t, 0.5)
    eps_t = consts.tile([128, 1], F32, tag="eps")
    nc.vector.memset(eps_t, EPS_S)
    bq_t = consts.tile([128, H], F32, tag="bq")
    nc.sync.dma_start(out=bq_t, in_=bq_d)
    ones2 = ones2_t[:, :, 0:1]

    def dma4(dst, src):
        nc.sync.dma_start(out=dst.rearrange("p a b c -> p (a b c)"),
                          in_=src.rearrange("p a b c -> p (a b c)"))

    def load_w(w_d, nm, shape):
        ts = []
        for p in range(NP):
            t = w_pool.tile(shape, FP8, tag=f"{nm}{p}", name=f"{nm}{p}")
            dma4(t, w_d[p])
            ts.append(t)
        return ts

    wk_t = load_w(wk_d, "wk", [128, H, 2, 128])
    xtm = []
    for p in range(NP):
        tm = xt_pool.tile([128, 4, 2, 512], FP8, tag=f"xtm{p}",
                          name=f"xtm{p}")
        dma4(tm, xtm_d[p])
        xtm.append(tm)
    wv_t = load_w(wv_d, "wv", [128, 2, 2, 512])
    wq_t = load_w(wq_d, "wq", [128, H, 2, 128])
    wo_t = load_w(wo_d, "wo", [128, 2, 2, 512])

    def xts(p, c):
        """V-proj stationary: [128, 2, 128] view of token chunk c."""
        return xtm[p][:, c // 4, :, (c % 4) * 128:(c % 4 + 1) * 128]

    gb = None
    if apply_gb:
        def bcast128(name, src):
            t = consts.tile([128, D], F32, tag=name, name=name)
            src_b = bass.AP(tensor=src.tensor, offset=src.offset,
                            ap=[[0, 128]] + src.ap)
            nc.sync.dma_start(out=t, in_=src_b)
            return t
        gb = [bcast128("gamma_b", gamma), bcast128("beta_b", beta)]

    kts = [kts_pool.tile([128, T], FP8, tag=f"kts{h}", name=f"kts{h}")
           for h in range(H)]
    qts = [qts_pool.tile([128, TQ], FP8, tag=f"qts{h}", name=f"qts{h}")
           for h in range(H)]
    vp = [vp_pool.tile([128, H, 2, 128], FP8, tag=f"vp{c}", name=f"vp{c}")
          for c in range(CP)]
    ctx4 = [ctx4_pool.tile([128, QC, 2, 128], FP8, tag=f"ctx{p}",
                           name=f"ctx{p}")
            for p in range(NP)]

    # ---- projection helpers ----------------------------------------------
    def proj_unit_K(h, nt):
        nsl = slice(nt * 512, (nt + 1) * 512)
        pp = ps512.tile([128, 512], F32, tag="ps", name="ppk")
        for p in range(NP):
            nc.tensor.matmul(pp, wk_t[p][:, h], xtm[p][:, nt],
                             start=(p == 0), stop=(p == NP - 1),
                             perf_mode=DR)
        nc.vector.tensor_copy(out=kts[h][:, nsl], in_=pp)

    def proj_unit_Q(h, nt):
        nsl = slice(nt * 512, (nt + 1) * 512)
        pp = ps512.tile([128, 512], F32, tag="ps", name="ppq")
        for p in range(NP):
            nc.tensor.matmul(pp, wq_t[p][:, h], xtm[p][:, nt],
                             start=(p == 0), stop=(p == NP - 1),
                             perf_mode=DR)
        nc.vector.tensor_scalar(out=qts[h][:, nsl], in0=pp,
                                scalar1=bq_t[:, h:h + 1], scalar2=None,
                                op0=ALU.add)

    def head_tasks(h):
        return ([lambda nt=nt: proj_unit_K(h, nt) for nt in range(4)]
                + [lambda nt=nt: proj_unit_Q(h, nt) for nt in range(2)])

    # ---- phase A: head-0 projections -------------------------------------
    for t in head_tasks(0):
        t()

    # score/exp issue machinery (runs ahead of the PV consumer)
    steps = [(nq, h, cp) for nq in range(NQ) for h in range(H)
             for cp in range(CP)]
    pt_q = {}
    cursor = [0]

    # In the nq=1 pass ScalarE paces the attention (2 EXPs/step > PE
    # work) while the DVE has slack, so alternating steps offload their
    # second exp chunk to the DVE as a Schraudolph fp8 exp:
    # bits8(e4m3) = round(logit * 8*log2e + 55.63)
    SCH_A = 8.0 * 1.4426950408889634 * SC_EXP
    SCH_B = 55.63
    I8 = mybir.dt.int8

    def issue_scores():
        i = cursor[0]
        nq, h, cp = steps[i]
        nsl = slice(nq * 512, (nq + 1) * 512)
        pt = pt_pool.tile([128, 2, 512], FP8, tag="pt", name="pt")
        sps = []
        for j in range(2):
            kc = cp * 2 + j
            s_ps = ps512.tile([128, 512], F32, tag="ps", name="s_ps")
            nc.tensor.matmul(s_ps, kts[h][:, kc * 128:(kc + 1) * 128],
                             qts[h][:, nsl], start=True, stop=True)
            sps.append(s_ps)
        nc.scalar.activation(out=pt[:, 0, :], in_=sps[0], func=AF.Exp,
                             scale=SC_EXP)
        # ScalarE paces wherever the projection drip has run dry: the
        # whole nq=1 pass, and nq=0 steps past the last drip pop (i=41)
        if (nq == 1 or i >= 44) and i % 2 == 1:
            nc.vector.tensor_scalar(out=pt[:, 1, :].bitcast(I8),
                                    in0=sps[1], scalar1=SCH_A,
                                    scalar2=SCH_B, op0=ALU.mult,
                                    op1=ALU.add)
        else:
            nc.scalar.activation(out=pt[:, 1, :], in_=sps[1], func=AF.Exp,
                                 scale=SC_EXP)
        pt_q[i] = pt
        cursor[0] += 1

    # ---- phase C/D machinery ---------------------------------------------
    xr = {}

    def fetch_xr(qc):
        t = xr_pool.tile([128, D], F32, tag="xr", name="xr")
        nc.sync.dma_start(out=t, in_=xq_d[qc * 128:(qc + 1) * 128, :])
        xr[qc] = t

    def normalize(nq, h, ctx_ps, sum_ps):
        rsum = sums_pool.tile([1, 512], F32, tag="rsum", name="rsum")
        nc.vector.reciprocal_approx_fast(out=rsum, in_=sum_ps)
        rsum_b = sums_pool.tile([128, 512], F32, tag="rsum_b",
                                name="rsum_b")
        nc.gpsimd.partition_broadcast(rsum_b, rsum, channels=128)
        nc.vector.tensor_mul(
            out=ctx4[h // 2][:, 4 * nq:4 * nq + 4, h % 2, :],
            in0=ctx_ps.rearrange("p (a b) -> p a b", b=128),
            in1=rsum_b.rearrange("p (a b) -> p a b", b=128))

    y1s, mvs = {}, {}

    def outproj_stage1(qc):
        """Out-proj matmuls + residual + LN stats (no ScalarE table use
        so it can drip between EXPs without activation-table thrash)."""
        if qc + 4 < QC:
            fetch_xr(qc + 4)
        y1 = y1_pool.tile([128, D], F32, tag="y1", name="y1")
        for n2 in range(2):
            n2sl = slice(n2 * 512, (n2 + 1) * 512)
            pp = ps512.tile([128, 512], F32, tag="ps", name="ppo")
            for p in range(NP):
                nc.tensor.matmul(pp, ctx4[p][:, qc], wo_t[p][:, n2],
                                 start=(p == 0), stop=(p == NP - 1),
                                 perf_mode=DR)
            nc.vector.tensor_add(out=y1[:, n2sl], in0=pp,
                                 in1=xr[qc][:, n2sl])
        xr.pop(qc)
        stats = ln_pool.tile([128, 2, 6], F32, tag="stats", name="stats")
        y1g = y1.rearrange("p (n f) -> p n f", f=512)
        nc.vector.bn_stats(out=stats[:, 0, :], in_=y1g[:, 0, :])
        nc.vector.bn_stats(out=stats[:, 1, :], in_=y1g[:, 1, :])
        mv = ln_pool.tile([128, 2], F32, tag="mv", name="mv")
        nc.vector.bn_aggr(out=mv, in_=stats)
        y1s[qc], mvs[qc] = y1, mv

    def outproj_stage2(qc):
        """Sqrt + normalize + store. Phase D has no EXPs, so Sqrts keep
        the activation table cached; the apply alternates ScalarE/DVE."""
        qs = slice(qc * 128, (qc + 1) * 128)
        y1, mv = y1s.pop(qc), mvs.pop(qc)
        std = ln_pool.tile([128, 1], F32, tag="std", name="std")
        nc.scalar.activation(out=std, in_=mv[:, 1:2], func=AF.Sqrt,
                             bias=eps_t)
        rstd = ln_pool.tile([128, 1], F32, tag="rstd", name="rstd")
        nc.vector.reciprocal(out=rstd, in_=std)
        y2 = y2_pool.tile([128, D], F32, tag="y2", name="y2")
        if apply_gb or qc % 2 == 1:
            nc.vector.tensor_scalar(out=y2, in0=y1, scalar1=mv[:, 0:1],
                                    scalar2=rstd, op0=ALU.subtract,
                                    op1=ALU.mult)
            if apply_gb:
                nc.vector.tensor_mul(out=y2, in0=y2, in1=gb[0])
                nc.vector.tensor_add(out=y2, in0=y2, in1=gb[1])
        else:
            # y2 = Identity(y1 * rstd + (-mu * rstd)) on the idle ScalarE
            nmr = ln_pool.tile([128, 1], F32, tag="nmr", name="nmr")
            nc.vector.tensor_scalar(out=nmr, in0=mv[:, 0:1], scalar1=rstd,
                                    scalar2=-1.0, op0=ALU.mult,
                                    op1=ALU.mult)
            nc.scalar.activation(out=y2, in_=y1, func=AF.Identity,
                                 bias=nmr, scale=rstd)
        nc.sync.dma_start(out=y[qs, :], in_=y2)

    # ---- phase B: V-proj interleaved with head-0 scores ------------------
    with tc.tile_pool(name="psv", bufs=2, space="PSUM") as psV:
        for c in range(2 * CP):
            ppv = psV.tile([128, D], F32, tag="psv", name="ppv")
            for half in range(2):
                n2sl = slice(half * 512, (half + 1) * 512)
                for p in range(NP):
                    nc.tensor.matmul(ppv[:, n2sl], xts(p, c),
                                     wv_t[p][:, half],
                                     start=(p == 0),
                                     stop=(p == NP - 1), perf_mode=DR)
            nc.vector.tensor_copy(
                out=vp[c // 2][:, :, c % 2, :],
                in_=ppv.rearrange("p (h m) -> p h m", m=128))
            if cursor[0] < CP:
                issue_scores()

    for qc in range(4):
        fetch_xr(qc)

    # ---- phase C: attention (out-proj stage1 drips through pass 1) -------
    drip = []
    for h in range(1, H):
        drip.extend(head_tasks(h))
    post = [lambda qc=qc: outproj_stage1(qc) for qc in range(4)]

    with tc.tile_pool(name="ctxps", bufs=2, space="PSUM") as ctx_pool, \
         tc.tile_pool(name="sumps", bufs=2, space="PSUM") as sum_pool:
        state = {}

        def attention_step(i):
            nq, h, cp = steps[i]
            while cursor[0] <= min(i + 2, len(steps) - 1):
                issue_scores()
            if cp == 0:
                state['ctx'] = ctx_pool.tile([128, 512], F32, tag="ctx",
                                             name="ctx_ps")
                state['sum'] = sum_pool.tile([1, 512], F32, tag="sum",
                                             name="sum_ps")
            pt = pt_q.pop(i)
            nc.tensor.matmul(state['ctx'], vp[cp][:, h], pt,
                             start=(cp == 0), stop=(cp == CP - 1),
                             perf_mode=DR)
            nc.tensor.matmul(state['sum'], ones2, pt,
                             start=(cp == 0), stop=(cp == CP - 1),
                             perf_mode=DR)
            if nq == 0 and drip:
                drip.pop(0)()
            elif nq == 1 and post and (i - CP * H) % 12 == 6:
                post.pop(0)()
            if cp == CP - 1:
                normalize(nq, h, state['ctx'], state['sum'])

        for i in range(len(steps)):
            attention_step(i)

    # ---- phase D: remaining out-projection q-chunks + LN epilogue --------
    # interleave: qc 0..3 finished stage1 during the nq=1 pass, so their
    # epilogue starts immediately while qc 4..7's matmuls run
    while post:
        post.pop(0)()
    for k in range(4):
        outproj_stage1(4 + k)
        outproj_stage2(k)
    for qc in range(4, QC):
        outproj_stage2(qc)


def build(apply_gb=True):
    nc = bacc.Bacc("TRN2", target_bir_lowering=False, debug=False,
                   enable_asserts=False, num_devices=N_CORES)
    ap = {}
    ap['xtm'] = nc.dram_tensor("xtm", [NP, 128, 4, 2, 512], FP8,
                               kind="ExternalInput").ap()
    for nm in ('wq', 'wk'):
        ap[nm] = nc.dram_tensor(nm, [NP, 128, H, 2, 128], FP8,
                                kind="ExternalInput").ap()
    for nm in ('wv', 'wo'):
        ap[nm] = nc.dram_tensor(nm, [NP, 128, 2, 2, 512], FP8,
                                kind="ExternalInput").ap()
    ap['bq'] = nc.dram_tensor("bq", [128, H], F32, kind="ExternalInput").ap()
    ap['xq'] = nc.dram_tensor("xq", [TQ, D], F32, kind="ExternalInput").ap()
    ap['gamma'] = nc.dram_tensor("gamma", [D], F32,
                                 kind="ExternalInput").ap()
    ap['beta'] = nc.dram_tensor("beta", [D], F32, kind="ExternalInput").ap()
    ap['y'] = nc.dram_tensor("y", [TQ, D], F32, kind="ExternalOutput").ap()

    with tile.TileContext(nc) as tc, contextlib.ExitStack() as es:
        _body(nc, tc, ap, es, apply_gb)
    nc.compile()
    return nc


def _pack_pairs(w8, inner):
    """[D, N] fp8 -> [NP, 128, N//inner, 2, inner]: row (2p+j)*128+r,
    col (o*inner+m) lands at [p, r, o, j, m] (k-plane pairs contiguous)."""
    n = w8.shape[1]
    return np.ascontiguousarray(
        w8.reshape(NP, 2, 128, n // inner, inner).transpose(0, 2, 3, 1, 4))


def make_in_maps(inputs):
    """Per-core input maps; x token-rotated so q tokens come first."""
    f32 = {k: np.asarray(v, dtype=np.float32) for k, v in inputs.items()}

    def w8(nm):
        return (f32[nm] * WS).astype(F8)

    shared = {
        'wq': _pack_pairs(w8('Wq'), 128),
        'wk': _pack_pairs(w8('Wk'), 128),
        'wv': _pack_pairs(w8('Wv'), 512),
        'wo': _pack_pairs(w8('Wo'), 512),
        'bq': np.ascontiguousarray(
            (WS * f32['bq']).reshape(H, 128).T.astype(np.float32)),
        'gamma': f32['gamma'],
        'beta': f32['beta'],
    }
    resid_c = f32['bo'] + f32['bv'] @ f32['Wo']
    x = f32['x']
    in_maps = []
    for core in range(N_CORES):
        b, g = divmod(core, 2)
        xr = np.roll(x[b], -TQ * g, axis=0)
        xt8 = xr.T.astype(F8)  # [D, T]
        in_maps.append({
            'xtm': _pack_pairs(xt8, 512),
            'xq': np.ascontiguousarray(XQS * (xr[:TQ] + resid_c)),
            **shared})
    return in_maps


_NC = None
_NC_GB = None


def kernel(**inputs):
    global _NC, _NC_GB
    apply_gb = not (np.all(np.asarray(inputs['gamma']) == 1.0)
                    and np.all(np.asarray(inputs['beta']) == 0.0))
    if _NC is None or _NC_GB != apply_gb:
        _NC = build(apply_gb)
        _NC_GB = apply_gb
    in_maps = make_in_maps(inputs)
    res = bass_utils.run_bass_kernel_spmd(_NC, in_maps,
                                          core_ids=list(range(N_CORES)))
    out = np.empty((B, T, D), dtype=np.float32)
    for core in range(N_CORES):
        b, g = divmod(core, 2)
        out[b, TQ * g:TQ * (g + 1)] = res.results[core]['y']
    return out


# revision 54
# speedup vs baseline: 1.0507x; 1.0128x over previous
"""Multi-head attention block (QKV proj + softmax attention + out-proj +
residual + LayerNorm) on 8 TRN2 NeuronCores.

Sharding: core = (batch b, token-half g). Each core computes K/V for the
FULL 2048 tokens of its batch locally (all 8 heads) -- no collectives at
all -- and runs attention + out-proj + LayerNorm for its 1024 query
tokens. Host rotates tokens per core so the core's query tokens are
always columns 0..1023 (softmax over k is permutation-invariant).

Precision: weights and x are pre-quantized to fp8 (e4m3) on the host
(weights pre-scaled x32 for mantissa range); projection / PV / sum /
out-proj matmuls run in fp8 DoubleRow perf mode (256-deep contraction
per instruction = 2x bf16 rate). DoubleRow operand slices keep their two
k-planes >=16B apart (ISA rule 's3_lw_dual_fp8_restrictions'). Scores
(contraction 128) run plain fp8. PSUM accumulation is fp32; softmax
statistics and LayerNorm are fp32.

Schedule: attention runs q-slice (nq) outer, head inner, software-
pipelined (scores+exp issue 2 steps ahead of PV). The V projection is
fused with head-0's first pass -- PV for chunk pair cp only needs
V-chunks 2cp/2cp+1, so each pair of V-proj chunks is followed directly
by one attention step. Later heads' K/Q projections drip one unit per
step through the nq=0 pass; the out-projection + LN stats for q-chunks
0..3 drip through the nq=1 pass; LN Sqrts all run back-to-back in the
epilogue (one activation-table load).

Bias algebra (host-folded): bk drops out of softmax entirely (adds a
per-query constant to every logit); bv's contribution to ctx is exactly
bv (softmax weights sum to 1) so bv@Wo + bo folds into the residual
term; only bq survives in-kernel (added to Q). The residual is
pre-scaled x2048 to absorb all fp8 weight scales -- LayerNorm is
scale-invariant so only eps needs adjusting.
"""

import contextlib
import sys

if '/opt/trn_rl_repo' not in sys.path:
    sys.path.insert(0, '/opt/trn_rl_repo')

import ml_dtypes
import numpy as np

import concourse.bacc as bacc
import concourse.bass as bass
import concourse.bass_utils as bass_utils
import concourse.tile as tile
from concourse import mybir

B, T, D, H = 4, 2048, 1024, 8
DH = 128
TQ = T // 2
N_CORES = 8
NP = 4              # d-chunk pairs (contraction 1024 = 4 x 256)
CP = 8              # k-chunk pairs (2048 = 8 x 256)
NQ = 2              # 512-wide q slices per core
QC = TQ // 128
EPS = 1e-5
WS = 32.0           # fp8 weight pre-scale
XQS = 2048.0        # residual pre-scale (= ctx-scale 64 x Wo-scale 32)
SC_EXP = 1.0 / (WS * WS * float(np.sqrt(DH)))
EPS_S = XQS * XQS * EPS
F32 = mybir.dt.float32
BF16 = mybir.dt.bfloat16
FP8 = mybir.dt.float8e4
AF = mybir.ActivationFunctionType
ALU = mybir.AluOpType
DR = mybir.MatmulPerfMode.DoubleRow
BF = ml_dtypes.bfloat16
F8 = ml_dtypes.float8_e4m3


def _body(nc, tc, ap, es, apply_gb):
    xtm_d = ap['xtm']
    wq_d, wk_d, wv_d, wo_d = ap['wq'], ap['wk'], ap['wv'], ap['wo']
    bq_d, xq_d, gamma, beta, y = (ap['bq'], ap['xq'], ap['gamma'],
                                  ap['beta'], ap['y'])

    consts = es.enter_context(tc.tile_pool(name="consts", bufs=1))
    xt_pool = es.enter_context(tc.tile_pool(name="xt", bufs=1))
    w_pool = es.enter_context(tc.tile_pool(name="w", bufs=1))
    kts_pool = es.enter_context(tc.tile_pool(name="kts", bufs=1))
    qts_pool = es.enter_context(tc.tile_pool(name="qts", bufs=1))
    vp_pool = es.enter_context(tc.tile_pool(name="vp", bufs=1))
    ctx4_pool = es.enter_context(tc.tile_pool(name="ctx4", bufs=1))
    pt_pool = es.enter_context(tc.tile_pool(name="pt", bufs=12))
    sums_pool = es.enter_context(tc.tile_pool(name="sums", bufs=3))
    xr_pool = es.enter_context(tc.tile_pool(name="xr", bufs=4))
    y2_pool = es.enter_context(tc.tile_pool(name="y2", bufs=2))
    ln_pool = es.enter_context(tc.tile_pool(name="ln", bufs=8))
    y1_pool = es.enter_context(tc.tile_pool(name="y1p", bufs=8))
    ps512 = es.enter_context(tc.tile_pool(name="ps512", bufs=4,
                                          space="PSUM"))

    # ---- constants & weights (DMA order = need order) --------------------
    # dual-fp8 LDWEIGHTS needs the two k-planes >=16B apart; pad to 16
    ones2_t = consts.tile([128, 2, 16], FP8, tag="ones2")
    nc.vector.memset(ones2_t, 0.5)
    eps_t = consts.tile([128, 1], F32, tag="eps")
    nc.vector.memset(eps_t, EPS_S)
    bq_t = consts.tile([128, H], F32, tag="bq")
    nc.sync.dma_start(out=bq_t, in_=bq_d)
    ones2 = ones2_t[:, :, 0:1]

    def dma4(dst, src):
        nc.sync.dma_start(out=dst.rearrange("p a b c -> p (a b c)"),
                          in_=src.rearrange("p a b c -> p (a b c)"))

    def load_w(w_d, nm, shape):
        ts = []
        for p in range(NP):
            t = w_pool.tile(shape, FP8, tag=f"{nm}{p}", name=f"{nm}{p}")
            dma4(t, w_d[p])
            ts.append(t)
        return ts

    wk_t = load_w(wk_d, "wk", [128, H, 2, 128])
    xtm = []
    for p in range(NP):
        tm = xt_pool.tile([128, 4, 2, 512], FP8, tag=f"xtm{p}",
                          name=f"xtm{p}")
        dma4(tm, xtm_d[p])
        xtm.append(tm)
    wv_t = load_w(wv_d, "wv", [128, 2, 2, 512])
    wq_t = load_w(wq_d, "wq", [128, H, 2, 128])
    wo_t = load_w(wo_d, "wo", [128, 2, 2, 512])

    def xts(p, c):
        """V-proj stationary: [128, 2, 128] view of token chunk c."""
        return xtm[p][:, c // 4, :, (c % 4) * 128:(c % 4 + 1) * 128]

    gb = None
    if apply_gb:
        def bcast128(name, src):
            t = consts.tile([128, D], F32, tag=name, name=name)
            src_b = bass.AP(tensor=src.tensor, offset=src.offset,
                            ap=[[0, 128]] + src.ap)
            nc.sync.dma_start(out=t, in_=src_b)
            return t
        gb = [bcast128("gamma_b", gamma), bcast128("beta_b", beta)]

    kts = [kts_pool.tile([128, T], FP8, tag=f"kts{h}", name=f"kts{h}")
           for h in range(H)]
    qts = [qts_pool.tile([128, TQ], FP8, tag=f"qts{h}", name=f"qts{h}")
           for h in range(H)]
    vp = [vp_pool.tile([128, H, 2, 128], FP8, tag=f"vp{c}", name=f"vp{c}")
          for c in range(CP)]
    ctx4 = [ctx4_pool.tile([128, QC, 2, 128], FP8, tag=f"ctx{p}",
                           name=f"ctx{p}")
            for p in range(NP)]

    # ---- projection helpers ----------------------------------------------
    def proj_unit_K(h, nt):
        nsl = slice(nt * 512, (nt + 1) * 512)
        pp = ps512.tile([128, 512], F32, tag="ps", name="ppk")
        for p in range(NP):
            nc.tensor.matmul(pp, wk_t[p][:, h], xtm[p][:, nt],
                             start=(p == 0), stop=(p == NP - 1),
                             perf_mode=DR)
        nc.vector.tensor_copy(out=kts[h][:, nsl], in_=pp)

    def proj_unit_Q(h, nt):
        nsl = slice(nt * 512, (nt + 1) * 512)
        pp = ps512.tile([128, 512], F32, tag="ps", name="ppq")
        for p in range(NP):
            nc.tensor.matmul(pp, wq_t[p][:, h], xtm[p][:, nt],
                             start=(p == 0), stop=(p == NP - 1),
                             perf_mode=DR)
        nc.vector.tensor_scalar(out=qts[h][:, nsl], in0=pp,
                                scalar1=bq_t[:, h:h + 1], scalar2=None,
                                op0=ALU.add)

    def head_tasks(h):
        return ([lambda nt=nt: proj_unit_K(h, nt) for nt in range(4)]
                + [lambda nt=nt: proj_unit_Q(h, nt) for nt in range(2)])

    # ---- phase A: head-0 projections -------------------------------------
    for t in head_tasks(0):
        t()

    # score/exp issue machinery (runs ahead of the PV consumer)
    steps = [(nq, h, cp) for nq in range(NQ) for h in range(H)
             for cp in range(CP)]
    pt_q = {}
    cursor = [0]

    # In the nq=1 pass ScalarE paces the attention (2 EXPs/step > PE
    # work) while the DVE has slack, so alternating steps offload their
    # second exp chunk to the DVE as a Schraudolph fp8 exp:
    # bits8(e4m3) = round(logit * 8*log2e + 55.63)
    SCH_A = 8.0 * 1.4426950408889634 * SC_EXP
    SCH_B = 55.63
    I8 = mybir.dt.int8

    def issue_scores():
        i = cursor[0]
        nq, h, cp = steps[i]
        nsl = slice(nq * 512, (nq + 1) * 512)
        pt = pt_pool.tile([128, 2, 512], FP8, tag="pt", name="pt")
        sps = []
        for j in range(2):
            kc = cp * 2 + j
            s_ps = ps512.tile([128, 512], F32, tag="ps", name="s_ps")
            nc.tensor.matmul(s_ps, kts[h][:, kc * 128:(kc + 1) * 128],
                             qts[h][:, nsl], start=True, stop=True)
            sps.append(s_ps)
        nc.scalar.activation(out=pt[:, 0, :], in_=sps[0], func=AF.Exp,
                             scale=SC_EXP)
        # ScalarE paces wherever the projection drip has run dry: the
        # whole nq=1 pass, and nq=0 steps past the last drip pop (i=41)
        if (nq == 1 or i >= 44) and i % 2 == 1:
            nc.vector.tensor_scalar(out=pt[:, 1, :].bitcast(I8),
                                    in0=sps[1], scalar1=SCH_A,
                                    scalar2=SCH_B, op0=ALU.mult,
                                    op1=ALU.add)
        else:
            nc.scalar.activation(out=pt[:, 1, :], in_=sps[1], func=AF.Exp,
                                 scale=SC_EXP)
        pt_q[i] = pt
        cursor[0] += 1

    # ---- phase C/D machinery ---------------------------------------------
    xr = {}

    def fetch_xr(qc):
        t = xr_pool.tile([128, D], F32, tag="xr", name="xr")
        nc.sync.dma_start(out=t, in_=xq_d[qc * 128:(qc + 1) * 128, :])
        xr[qc] = t

    def normalize(nq, h, ctx_ps, sum_ps):
        rsum = sums_pool.tile([1, 512], F32, tag="rsum", name="rsum")
        nc.vector.reciprocal_approx_fast(out=rsum, in_=sum_ps)
        rsum_b = sums_pool.tile([128, 512], F32, tag="rsum_b",
                                name="rsum_b")
        nc.gpsimd.partition_broadcast(rsum_b, rsum, channels=128)
        nc.vector.tensor_mul(
            out=ctx4[h // 2][:, 4 * nq:4 * nq + 4, h % 2, :],
            in0=ctx_ps.rearrange("p (a b) -> p a b", b=128),
            in1=rsum_b.rearrange("p (a b) -> p a b", b=128))

    y1s, mvs = {}, {}

    def outproj_half(qc, n2):
        """Half an out-proj q-chunk (4 matmuls + residual add); the
        second half adds the LN stats. Dripped in halves so the PE
        burst between attention steps stays short."""
        if n2 == 0:
            if qc + 4 < QC:
                fetch_xr(qc + 4)
            y1s[qc] = y1_pool.tile([128, D], F32, tag="y1", name="y1")
        y1 = y1s[qc]
        n2sl = slice(n2 * 512, (n2 + 1) * 512)
        pp = ps512.tile([128, 512], F32, tag="ps", name="ppo")
        for p in range(NP):
            nc.tensor.matmul(pp, ctx4[p][:, qc], wo_t[p][:, n2],
                             start=(p == 0), stop=(p == NP - 1),
                             perf_mode=DR)
        nc.vector.tensor_add(out=y1[:, n2sl], in0=pp,
                             in1=xr[qc][:, n2sl])
        if n2 == 1:
            xr.pop(qc)
            stats = ln_pool.tile([128, 2, 6], F32, tag="stats",
                                 name="stats")
            y1g = y1.rearrange("p (n f) -> p n f", f=512)
            nc.vector.bn_stats(out=stats[:, 0, :], in_=y1g[:, 0, :])
            nc.vector.bn_stats(out=stats[:, 1, :], in_=y1g[:, 1, :])
            mv = ln_pool.tile([128, 2], F32, tag="mv", name="mv")
            nc.vector.bn_aggr(out=mv, in_=stats)
            mvs[qc] = mv

    def outproj_stage1(qc):
        outproj_half(qc, 0)
        outproj_half(qc, 1)

    def outproj_stage2(qc):
        """Sqrt + normalize + store. Phase D has no EXPs, so Sqrts keep
        the activation table cached; the apply alternates ScalarE/DVE."""
        qs = slice(qc * 128, (qc + 1) * 128)
        y1, mv = y1s.pop(qc), mvs.pop(qc)
        std = ln_pool.tile([128, 1], F32, tag="std", name="std")
        nc.scalar.activation(out=std, in_=mv[:, 1:2], func=AF.Sqrt,
                             bias=eps_t)
        rstd = ln_pool.tile([128, 1], F32, tag="rstd", name="rstd")
        nc.vector.reciprocal(out=rstd, in_=std)
        y2 = y2_pool.tile([128, D], F32, tag="y2", name="y2")
        if apply_gb or qc % 2 == 1:
            nc.vector.tensor_scalar(out=y2, in0=y1, scalar1=mv[:, 0:1],
                                    scalar2=rstd, op0=ALU.subtract,
                                    op1=ALU.mult)
            if apply_gb:
                nc.vector.tensor_mul(out=y2, in0=y2, in1=gb[0])
                nc.vector.tensor_add(out=y2, in0=y2, in1=gb[1])
        else:
            # y2 = Identity(y1 * rstd + (-mu * rstd)) on the idle ScalarE
            nmr = ln_pool.tile([128, 1], F32, tag="nmr", name="nmr")
            nc.vector.tensor_scalar(out=nmr, in0=mv[:, 0:1], scalar1=rstd,
                                    scalar2=-1.0, op0=ALU.mult,
                                    op1=ALU.mult)
            nc.scalar.activation(out=y2, in_=y1, func=AF.Identity,
                                 bias=nmr, scale=rstd)
        nc.sync.dma_start(out=y[qs, :], in_=y2)

    # ---- phase B: V-proj interleaved with head-0 scores ------------------
    with tc.tile_pool(name="psv", bufs=2, space="PSUM") as psV:
        for c in range(2 * CP):
            ppv = psV.tile([128, D], F32, tag="psv", name="ppv")
            for half in range(2):
                n2sl = slice(half * 512, (half + 1) * 512)
                for p in range(NP):
                    nc.tensor.matmul(ppv[:, n2sl], xts(p, c),
                                     wv_t[p][:, half],
                                     start=(p == 0),
                                     stop=(p == NP - 1), perf_mode=DR)
            nc.vector.tensor_copy(
                out=vp[c // 2][:, :, c % 2, :],
                in_=ppv.rearrange("p (h m) -> p h m", m=128))
            if cursor[0] < CP:
                issue_scores()

    for qc in range(4):
        fetch_xr(qc)

    # ---- phase C: attention (out-proj stage1 drips through pass 1) -------
    drip = []
    for h in range(1, H):
        drip.extend(head_tasks(h))
    post = [lambda qc=qc, n2=n2: outproj_half(qc, n2)
            for qc in range(4) for n2 in range(2)]

    with tc.tile_pool(name="ctxps", bufs=2, space="PSUM") as ctx_pool, \
         tc.tile_pool(name="sumps", bufs=2, space="PSUM") as sum_pool:
        state = {}

        def attention_step(i):
            nq, h, cp = steps[i]
            while cursor[0] <= min(i + 2, len(steps) - 1):
                issue_scores()
            if cp == 0:
                state['ctx'] = ctx_pool.tile([128, 512], F32, tag="ctx",
                                             name="ctx_ps")
                state['sum'] = sum_pool.tile([1, 512], F32, tag="sum",
                                             name="sum_ps")
            pt = pt_q.pop(i)
            nc.tensor.matmul(state['ctx'], vp[cp][:, h], pt,
                             start=(cp == 0), stop=(cp == CP - 1),
                             perf_mode=DR)
            nc.tensor.matmul(state['sum'], ones2, pt,
                             start=(cp == 0), stop=(cp == CP - 1),
                             perf_mode=DR)
            if nq == 0 and drip:
                drip.pop(0)()
            elif nq == 1 and post and (i - CP * H) % 6 == 3:
                post.pop(0)()
            if cp == CP - 1:
                normalize(nq, h, state['ctx'], state['sum'])

        for i in range(len(steps)):
            attention_step(i)

    # ---- phase D: remaining out-projection q-chunks + LN epilogue --------
    # interleave: qc 0..3 finished stage1 during the nq=1 pass, so their
    # epilogue starts immediately while qc 4..7's matmuls run
    while post:
        post.pop(0)()
    for k in range(4):
        outproj_stage1(4 + k)
        outproj_stage2(k)
    for qc in range(4, QC):
        outproj_stage2(qc)


def build(apply_gb=True):
    nc = bacc.Bacc("TRN2", target_bir_lowering=False, debug=False,
                   enable_asserts=False, num_devices=N_CORES)
    ap = {}
    ap['xtm'] = nc.dram_tensor("xtm", [NP, 128, 4, 2, 512], FP8,
                               kind="ExternalInput").ap()
    for nm in ('wq', 'wk'):
        ap[nm] = nc.dram_tensor(nm, [NP, 128, H, 2, 128], FP8,
                                kind="ExternalInput").ap()
    for nm in ('wv', 'wo'):
        ap[nm] = nc.dram_tensor(nm, [NP, 128, 2, 2, 512], FP8,
                                kind="ExternalInput").ap()
    ap['bq'] = nc.dram_tensor("bq", [128, H], F32, kind="ExternalInput").ap()
    ap['xq'] = nc.dram_tensor("xq", [TQ, D], F32, kind="ExternalInput").ap()
    ap['gamma'] = nc.dram_tensor("gamma", [D], F32,
                                 kind="ExternalInput").ap()
    ap['beta'] = nc.dram_tensor("beta", [D], F32, kind="ExternalInput").ap()
    ap['y'] = nc.dram_tensor("y", [TQ, D], F32, kind="ExternalOutput").ap()

    with tile.TileContext(nc) as tc, contextlib.ExitStack() as es:
        _body(nc, tc, ap, es, apply_gb)
    nc.compile()
    return nc


def _pack_pairs(w8, inner):
    """[D, N] fp8 -> [NP, 128, N//inner, 2, inner]: row (2p+j)*128+r,
    col (o*inner+m) lands at [p, r, o, j, m] (k-plane pairs contiguous)."""
    n = w8.shape[1]
    return np.ascontiguousarray(
        w8.reshape(NP, 2, 128, n // inner, inner).transpose(0, 2, 3, 1, 4))


def make_in_maps(inputs):
    """Per-core input maps; x token-rotated so q tokens come first."""
    f32 = {k: np.asarray(v, dtype=np.float32) for k, v in inputs.items()}

    def w8(nm):
        return (f32[nm] * WS).astype(F8)

    shared = {
        'wq': _pack_pairs(w8('Wq'), 128),
        'wk': _pack_pairs(w8('Wk'), 128),
        'wv': _pack_pairs(w8('Wv'), 512),
        'wo': _pack_pairs(w8('Wo'), 512),
        'bq': np.ascontiguousarray(
            (WS * f32['bq']).reshape(H, 128).T.astype(np.float32)),
        'gamma': f32['gamma'],
        'beta': f32['beta'],
    }
    resid_c = f32['bo'] + f32['bv'] @ f32['Wo']
    x = f32['x']
    in_maps = []
    for core in range(N_CORES):
        b, g = divmod(core, 2)
        xr = np.roll(x[b], -TQ * g, axis=0)
        xt8 = xr.T.astype(F8)  # [D, T]
        in_maps.append({
            'xtm': _pack_pairs(xt8, 512),
            'xq': np.ascontiguousarray(XQS * (xr[:TQ] + resid_c)),
            **shared})
    return in_maps


_NC = None
_NC_GB = None


def kernel(**inputs):
    global _NC, _NC_GB
    apply_gb = not (np.all(np.asarray(inputs['gamma']) == 1.0)
                    and np.all(np.asarray(inputs['beta']) == 0.0))
    if _NC is None or _NC_GB != apply_gb:
        _NC = build(apply_gb)
        _NC_GB = apply_gb
    in_maps = make_in_maps(inputs)
    res = bass_utils.run_bass_kernel_spmd(_NC, in_maps,
                                          core_ids=list(range(N_CORES)))
    out = np.empty((B, T, D), dtype=np.float32)
    for core in range(N_CORES):
        b, g = divmod(core, 2)
        out[b, TQ * g:TQ * (g + 1)] = res.results[core]['y']
    return out
